# revision 1
# baseline (speedup 1.0000x reference)
"""TRN2 Bass kernel for nn_MetaBaseline (DN4-style local-descriptor kNN).

Reference computation (per batch b):
  q = normalize(input1[b].reshape(75, 100, 640), axis=-1)      # query patches
  s = normalize(input2[b].reshape(2500, 640), axis=-1)         # support descs
  scores = q_patches @ s.T                                     # [7500, 2500]
  per way group g (columns [500g, 500g+500)): top-k per row, mean,
  then sum over the 100 patches of each query -> out [75, 5].

Sharding: data-parallel over (b, query-quarter): 8 cores, each handles one
batch's quarter of queries (19 queries padded) with that batch's full
support replicated.

Per-core device program. Engines execute in emission order, so emission is
software-pipelined. The score loop is WAY-OUTER: pass w only needs support
descriptor tiles 0..4w+3, so score matmuls start as soon as the first four
support tiles are normalized+transposed; the remaining support prep streams
in the background during passes 0-3, and query prep (norm chain, packed PE
transposes, float32r eviction) is folded into pass 0 one tile ahead.
Top-8 per (patch, way) via DVE max straight from the PSUM score bank
(bank freed immediately after); pass 4 finishes each patch tile with a
strided top-k tensor_reduce, ACT scale by 1/(k*|q_patch|), and a small
fp32 indicator matmul accumulating per-query sums in PSUM -> [19, 5].
"""
import os
from contextlib import ExitStack

import numpy as np

import concourse.bass as bass  # noqa: F401
import concourse.mybir as mybir
import concourse.tile as tile
from concourse import bacc
from concourse.bass_utils import run_bass_kernel_spmd

# Problem geometry (hardcoded per contest rules)
B, Q, WAY, SHOT, H, W, C = 2, 75, 5, 5, 10, 10, 640
HW = H * W               # 100 patches per query / support image
NQ = 19                  # queries per core (4 cores x 19 = 76 >= 75)
MT = 15                  # patch M-tiles of 128 -> 1920 rows (1900 real)
PAD_P = MT * 128
NS = WAY * SHOT * HW     # 2500 support descriptors per batch
ST = 20                  # support tiles of 128 -> 2560 rows
PAD_S = ST * 128
KC = 5                   # C chunks of 128 (640 = 5*128)
P = 128
NW = SHOT * HW           # 500 support descriptors per way group
N_CORES = 8
N_WARM = int(os.environ.get("N_WARM", "32"))
BF16 = os.environ.get("BF16", "0") == "1"  # experimental: bf16 score operands

_prog_cache: dict[int, object] = {}


def _build(k: int):
    """Build + compile the per-core SPMD program for neighbor_k == k."""
    assert 1 <= k <= 8, f"neighbor_k={k} not supported (need 1..8)"
    nc = bacc.Bacc("TRN2", target_bir_lowering=False, debug=False)
    f32 = mybir.dt.float32
    f32r = mybir.dt.float32r
    t_dt = mybir.dt.bfloat16 if BF16 else f32r
    AF = mybir.ActivationFunctionType

    q_d = nc.dram_tensor("q", [PAD_P, C], f32, kind="ExternalInput").ap()
    s_d = nc.dram_tensor("s", [PAD_S, C], f32, kind="ExternalInput").ap()
    ind_d = nc.dram_tensor("ind", [P, MT * NQ], f32, kind="ExternalInput").ap()
    ident_d = nc.dram_tensor("ident", [P, P], f32, kind="ExternalInput").ap()
    out_d = nc.dram_tensor("out", [NQ, WAY], f32, kind="ExternalOutput").ap()

    with tile.TileContext(nc) as tc:
        with ExitStack() as ctx:
            const = ctx.enter_context(tc.tile_pool(name="const", bufs=1))
            big = ctx.enter_context(tc.tile_pool(name="big", bufs=1))
            loads = ctx.enter_context(tc.tile_pool(name="loads", bufs=7))
            small = ctx.enter_context(tc.tile_pool(name="small", bufs=4))
            mxp = ctx.enter_context(tc.tile_pool(name="mxp", bufs=MT))
            outp = ctx.enter_context(
                tc.tile_pool(name="outp", bufs=1, space="PSUM")
            )
            tp4 = ctx.enter_context(
                tc.tile_pool(name="tp4", bufs=2, space="PSUM")
            )
            tp1 = ctx.enter_context(
                tc.tile_pool(name="tp1", bufs=2, space="PSUM")
            )
            spp = ctx.enter_context(
                tc.tile_pool(name="spp", bufs=3, space="PSUM")
            )

            ident = const.tile([P, P], f32)
            ident_r = const.tile([P, P], f32r, name="ident_r")
            ind_sb = const.tile([P, MT * NQ], f32)
            qinv = const.tile([P, MT], f32)

            # chunk c of each transposed tensor has its own column band so a
            # packed 4-chunk PSUM bank evicts with one strided copy
            s_T = big.tile([P, KC * PAD_S], t_dt, name="s_T")
            q_T = big.tile([P, KC * PAD_P], t_dt, name="q_T")

            def sT(c):
                return s_T[:, c * PAD_S:(c + 1) * PAD_S]

            def qT(c):
                return q_T[:, c * PAD_P:(c + 1) * PAD_P]

            out_ps = outp.tile([NQ, WAY], f32)

            # ---- warmups: ACT tables + PE pipeline (no DMA deps) ----
            wtile = const.tile([P, P], f32, name="wtile")
            nc.vector.memset(wtile, 1.0)
            wsq = small.tile([P, 1], f32, tag="snrm")
            nc.scalar.sqrt(wsq, wtile[:, 0:1])
            wps = tp4.tile([P, 4 * P], f32, tag="tp4")
            for i in range(N_WARM):
                nc.tensor.transpose(
                    wps[:, (i % 4) * P:(i % 4 + 1) * P], wtile, wtile
                )

            nev = 0

            def evict(out_ap, src_ap):
                nonlocal nev
                if nev % 2 == 0:
                    nc.vector.tensor_copy(out_ap, src_ap)
                else:
                    nc.scalar.copy(out_ap, src_ap)
                nev += 1

            def transpose_evict(x, T_all, T_pad, t, defer=False):
                """5 packed PE transposes of x into T_all's column bands."""
                isr = x.dtype == f32r
                idn = ident_r if isr else ident
                psA = tp4.tile([P, 4 * P], f32, tag="tp4", name=f"psA_{t}")
                psB = tp1.tile([P, P], f32, tag="tp1", name=f"psB_{t}")
                psAv = psA.bitcast(f32r) if isr else psA
                psBv = psB.bitcast(f32r) if isr else psB
                for c in range(4):
                    nc.tensor.transpose(
                        psAv[:, c * P:(c + 1) * P],
                        x[:, c * P:(c + 1) * P], idn)
                nc.tensor.transpose(psBv, x[:, 4 * P:5 * P], idn)
                out_ap = T_all[:, :4 * T_pad].rearrange(
                    "p (c n) -> p c n", c=4
                )[:, :, t * P:(t + 1) * P]

                def _ev():
                    evict(out_ap, psA.rearrange("p (c n) -> p c n", c=4))
                    evict(
                        T_all[:, 4 * T_pad + t * P:4 * T_pad + (t + 1) * P],
                        psB)
                if defer:
                    return _ev
                _ev()

            xs_s = [None] * ST
            xs_q = [None] * MT

            def s_dma(t, split=1):
                x = loads.tile([P, C], f32, tag="x_tile", name=f"sx{t}")
                h = P // split
                for i in range(split):
                    nc.sync.dma_start(
                        out=x[i * h:(i + 1) * h, :],
                        in_=s_d[t * P + i * h:t * P + (i + 1) * h, :])
                xs_s[t] = x

            def q_dma(m, split=1):
                x = loads.tile([P, C], f32, tag="x_tile", name=f"qx{m}")
                h = P // split
                for i in range(split):
                    nc.sync.dma_start(
                        out=x[i * h:(i + 1) * h, :],
                        in_=q_d[m * P + i * h:m * P + (i + 1) * h, :])
                xs_q[m] = x

            def s_prep(t, sq_on_dve=False, scale_on_dve=False,
                       defer=False):
                x = xs_s[t]
                sq = loads.tile([P, C], f32, tag="sq", name=f"ssq{t}")
                ssum = small.tile([P, 1], f32, tag="ssum")
                if sq_on_dve:
                    nc.vector.tensor_tensor_reduce(
                        sq, x, x, 1.0, 0.0,
                        mybir.AluOpType.mult, mybir.AluOpType.add, ssum)
                else:
                    nc.scalar.activation(sq, x, AF.Square, accum_out=ssum)
                snrm = small.tile([P, 1], f32, tag="snrm")
                nc.scalar.sqrt(snrm, ssum)
                sinv = small.tile([P, 1], f32, tag="sinv")
                nc.vector.reciprocal(sinv, snrm)
                s_n = loads.tile([P, C], f32r, tag="s_n", name=f"sn{t}")
                if scale_on_dve:
                    nc.vector.tensor_scalar_mul(s_n, x, sinv)
                else:
                    # NOTE: never gpsimd here - tensor_scalar on GPSIMD
                    # measures ~9.3us per [128,640] tile on real TRN2
                    nc.scalar.mul(s_n, x, sinv)
                return transpose_evict(s_n, s_T, PAD_S, t, defer=defer)

            def q_prep(m, defer=False):
                x = xs_q[m]
                ev = transpose_evict(x, q_T, PAD_P, m, defer=defer)
                sq = loads.tile([P, C], f32, tag="sq", name=f"qsq{m}")
                qsum = small.tile([P, 1], f32, tag="ssum")
                nc.scalar.activation(sq, x, AF.Square, accum_out=qsum)
                kn = small.tile([P, 1], f32, tag="snrm")
                # sqrt(k^2 * sum(q^2)) = k * |q|
                nc.scalar.activation(kn, qsum, AF.Sqrt, scale=float(k * k))
                nc.vector.reciprocal(qinv[:, m:m + 1], kn)
                return ev

            # ---- prologue: support tiles 0-3, queries 0-1 ----
            # DMA order: support first (its prep chain is the pace-setter),
            # then ident (first needed by real transposes), queries, ind.
            for t in range(4):
                s_dma(t)
            nc.sync.dma_start(out=ident, in_=ident_d)
            nc.vector.tensor_copy(ident_r, ident)
            q_dma(0)
            q_dma(1)
            nc.sync.dma_start(out=ind_sb, in_=ind_d)
            next_s = [4]

            def s_dma_ahead(upto):
                while next_s[0] <= min(upto, ST - 1):
                    s_dma(next_s[0])
                    next_s[0] += 1

            s_prep(0, scale_on_dve=True)
            s_prep(1, scale_on_dve=True)
            q_prep(0)
            s_prep(2, scale_on_dve=True)
            s_dma_ahead(5)
            s_prep(3, scale_on_dve=True)

            # s-prep schedule: pass w preps tiles 4w+4 .. 4w+7 (w<4)
            mxs = [None] * MT
            prev = [None, None]
            for w in range(WAY):
                for m in range(MT):
                    if w == 0:
                        if m + 2 < MT:
                            q_dma(m + 2)
                        if m + 1 < MT:
                            q_prep(m + 1)
                    if w < 4 and m in (1, 5, 9, 13):
                        t = 4 * (w + 1) + (m - 1) // 4
                        s_dma_ahead(t + 3)
                        s_prep(t, scale_on_dve=(t % 2 == 1))
                    if w == 0:
                        mxs[m] = mxp.tile([P, WAY * 8], f32, tag="mx",
                                          name=f"mx{m}")
                    psc = spp.tile([P, NW], f32, tag="psc",
                                   name=f"psc{m}_{w}")
                    for c in range(KC):
                        nc.tensor.matmul(
                            psc,
                            qT(c)[:, m * P:(m + 1) * P],
                            sT(c)[:, w * NW:(w + 1) * NW],
                            start=(c == 0),
                            stop=(c == KC - 1),
                        )
                    nc.vector.max(mxs[m][:, w * 8:(w + 1) * 8], psc)
                    if w == WAY - 1:
                        tsum = small.tile([P, WAY], f32, tag="tsum")
                        nc.vector.tensor_reduce(
                            tsum,
                            mxs[m].rearrange("p (w j) -> p w j", w=WAY)[:, :, :k],
                            axis=mybir.AxisListType.X,
                            op=mybir.AluOpType.add,
                        )
                        scaled = small.tile([P, WAY], f32, tag="scaled")
                        nc.scalar.mul(scaled, tsum, qinv[:, m:m + 1])
                        if prev[0] is not None:
                            nc.tensor.matmul(
                                out_ps,
                                ind_sb[:, prev[1] * NQ:(prev[1] + 1) * NQ],
                                prev[0], start=(prev[1] == 0), stop=False)
                        prev = [scaled, m]
            nc.tensor.matmul(
                out_ps, ind_sb[:, prev[1] * NQ:(prev[1] + 1) * NQ],
                prev[0], start=False, stop=True)
            out_sb = small.tile([NQ, WAY], f32, tag="out_sb")
            nc.scalar.copy(out_sb, out_ps)
            nc.sync.dma_start(out=out_d, in_=out_sb)

    nc.compile()
    return nc


def get_program(k: int):
    if k not in _prog_cache:
        _prog_cache[k] = _build(k)
    return _prog_cache[k]


def make_in_maps(input1: np.ndarray, input2: np.ndarray):
    """Shard full inputs into per-core input maps."""
    input1 = np.ascontiguousarray(np.asarray(input1), dtype=np.float32)
    input2 = np.ascontiguousarray(np.asarray(input2), dtype=np.float32)
    in_maps = []
    for core in range(N_CORES):
        b = core // 4
        qs = (core % 4) * NQ
        qe = min(Q, qs + NQ)
        nq = qe - qs
        qdat = input1[b].reshape(Q, HW, C)[qs:qe].reshape(-1, C)
        qfull = np.ones((PAD_P, C), np.float32)
        qfull[: nq * HW] = qdat
        sfull = np.ones((PAD_S, C), np.float32)
        sfull[:NS] = input2[b].reshape(NS, C)
        # indicator: patch row p of M-tile t belongs to query (t*128+p)//HW
        ind = np.zeros((P, MT * NQ), np.float32)
        g = np.arange(MT * P)
        j = g // HW
        valid = j < nq
        ind[g[valid] % P, (g[valid] // P) * NQ + j[valid]] = 1.0
        in_maps.append({"q": qfull, "s": sfull, "ind": ind,
                        "ident": np.eye(P, dtype=np.float32)})
    return in_maps


def gather_out(results) -> np.ndarray:
    out = np.zeros((B, Q, WAY), np.float32)
    for core in range(N_CORES):
        b = core // 4
        qs = (core % 4) * NQ
        n = min(Q, qs + NQ) - qs
        out[b, qs:qs + n] = results[core]["out"][:n]
    return out


def kernel(input1, input2, neighbor_k):
    k = int(np.asarray(neighbor_k))
    nc = get_program(k)
    in_maps = make_in_maps(input1, input2)
    # the axon-tunneled device occasionally reports a transient
    # "unrecoverable" state right after a previous process's teardown;
    # it recovers within seconds, so retry a couple of times
    import time
    last = None
    for attempt in range(3):
        try:
            res = run_bass_kernel_spmd(
                nc, in_maps, core_ids=list(range(N_CORES)))
            return gather_out(res.results)
        except Exception as e:  # noqa: BLE001
            last = e
            if attempt < 2:
                time.sleep(20.0 * (attempt + 1))
    raise last



# revision 2
# speedup vs baseline: 1.4747x; 1.4747x over previous
"""TRN2 Bass kernel for nn_MetaBaseline (DN4-style local-descriptor kNN).

Reference computation (per batch b):
  q = input1[b].reshape(7500, 640)           # query patch descriptors
  s = normalize(input2[b].reshape(2500, 640), axis=-1)
  scores = q @ s.T                           # [7500, 2500]
  per way group g (columns [500g, 500g+500)): top-k per row, mean,
  divide by |q_patch| (commutes with top-k since it is per-row), then
  sum over the 100 patches of each query -> out [75, 5].

Sharding: data-parallel over (b, query-quarter): 8 cores, each handles one
batch's quarter of queries (19 queries padded) with that batch's full
support replicated.

Implementation highlights vs the fp32r baseline:
- Score matmuls run in fp8e4m3 with MatmulPerfMode.DoubleRow (two 128-row
  k-tiles per pass): C=640 is padded to 6 chunks of 128 (band 5 zeros) so
  each (patch-tile, way) pair needs just 3 PE instructions.
- The query operand is uploaded pre-transposed in fp8 from the host
  (layout + dtype conversion only), so queries need no PE transpose, no
  PSUM eviction and no quantize pass on device. A second row-major bf16
  copy of q feeds the |q_patch| norm pass (ACT square+accum), matching the
  reference's exact-norm division.
- Support arrives row-major bf16; on device: ACT square+accum -> batched
  sqrt(scale=1/alpha^2) -> DVE reciprocal gives sinv = alpha/|s| per
  descriptor; DVE tensor_scalar (4x mode on bf16) or ACT scales to
  alpha-normalized bf16; PE transposes (bf16 identity, 1 cycle/row) into
  one PSUM bank per tile; a single strided copy evicts+converts to the
  fp8 banded layout. alpha=32 keeps e4m3 operands in the normal range;
  1/alpha folds into the per-query scale sqrt(k^2 alpha^2 sum q^2).
- Top-8 per (patch, way) via DVE max8 straight from the PSUM score bank;
  pass 4 finishes each patch tile with a strided top-k tensor_reduce, an
  ACT scale by 1/(k*alpha*|q_patch|), and a small fp32 indicator matmul
  accumulating per-query sums in PSUM -> [19, 5].
"""
import os
from contextlib import ExitStack

import ml_dtypes
import numpy as np

import concourse.bass as bass  # noqa: F401
import concourse.mybir as mybir
import concourse.tile as tile
from concourse import bacc
from concourse.bass_utils import run_bass_kernel_spmd

# Problem geometry (hardcoded per contest rules)
B, Q, WAY, SHOT, H, W, C = 2, 75, 5, 5, 10, 10, 640
HW = H * W               # 100 patches per query / support image
NQ = 19                  # queries per core (4 cores x 19 = 76 >= 75)
MT = 15                  # patch M-tiles of 128 -> 1920 rows (1900 real)
PAD_P = MT * 128
NS = WAY * SHOT * HW     # 2500 support descriptors per batch
ST = 20                  # support tiles of 128 -> 2560 rows
PAD_S = ST * 128
KC = 5                   # real C chunks of 128 (640 = 5*128)
KP = 3                   # DoubleRow chunk pairs (6 bands incl. zero band)
P = 128
NW = SHOT * HW           # 500 support descriptors per way group
N_CORES = 8
ALPHA = 32.0             # support operand scale for fp8 normal range
N_WARM = int(os.environ.get("N_WARM", "24"))

_prog_cache: dict[int, object] = {}


def _build(k: int):
    """Build + compile the per-core SPMD program for neighbor_k == k."""
    assert 1 <= k <= 8, f"neighbor_k={k} not supported (need 1..8)"
    nc = bacc.Bacc("TRN2", target_bir_lowering=False, debug=False)
    f32 = mybir.dt.float32
    bf16 = mybir.dt.bfloat16
    fp8 = mybir.dt.float8e4
    AF = mybir.ActivationFunctionType
    DR = mybir.MatmulPerfMode.DoubleRow

    qT_d = nc.dram_tensor("qT", [P, 2 * KP * PAD_P], fp8,
                          kind="ExternalInput").ap()
    qr_d = nc.dram_tensor("qr", [P, MT * C], bf16, kind="ExternalInput").ap()
    sr_d = nc.dram_tensor("sr", [P, ST * C], bf16, kind="ExternalInput").ap()
    zs_d = nc.dram_tensor("zs", [P, PAD_S], fp8, kind="ExternalInput").ap()
    ind_d = nc.dram_tensor("ind", [P, MT * NQ], f32, kind="ExternalInput").ap()
    ident_d = nc.dram_tensor("ident", [P, P], bf16, kind="ExternalInput").ap()
    out_d = nc.dram_tensor("out", [NQ, WAY], f32, kind="ExternalOutput").ap()

    with tile.TileContext(nc) as tc:
        with ExitStack() as ctx:
            const = ctx.enter_context(tc.tile_pool(name="const", bufs=1))
            big = ctx.enter_context(tc.tile_pool(name="big", bufs=1))
            loads = ctx.enter_context(tc.tile_pool(name="loads", bufs=8))
            work = ctx.enter_context(tc.tile_pool(name="work", bufs=3))
            small = ctx.enter_context(tc.tile_pool(name="small", bufs=4))
            mxp = ctx.enter_context(tc.tile_pool(name="mxp", bufs=MT))
            outp = ctx.enter_context(
                tc.tile_pool(name="outp", bufs=1, space="PSUM")
            )
            tpp = ctx.enter_context(
                tc.tile_pool(name="tpp", bufs=2, space="PSUM")
            )
            warmp = ctx.enter_context(
                tc.tile_pool(name="warmp", bufs=1, space="PSUM")
            )
            spp = ctx.enter_context(
                tc.tile_pool(name="spp", bufs=3, space="PSUM")
            )

            ident = const.tile([P, P], bf16)
            ind_sb = const.tile([P, MT * NQ], f32)
            # banded transposed operands: [partition, chunk band, column]
            s8T = big.tile([P, 2 * KP, PAD_S], fp8, name="s8T")
            qT_sb = big.tile([P, 2 * KP, PAD_P], fp8, name="qT_sb")
            qr_sb = big.tile([P, MT * C], bf16, name="qr_sb")

            # batched norm scalars
            ssum_all = const.tile([P, ST], f32, name="ssum")
            sinv_all = const.tile([P, ST], f32, name="sinv")
            qsum_all = const.tile([P, MT], f32, name="qsum")
            qinv = const.tile([P, MT], f32, name="qinv")

            out_ps = outp.tile([NQ, WAY], f32)

            # ---- warmups: ACT tables + PE pipeline + pstate ramp ----
            wtile = const.tile([P, P], bf16, name="wtile")
            nc.vector.memset(wtile, 1.0)
            wsq = small.tile([P, P], bf16, tag="wsq")
            wss = small.tile([P, 1], f32, tag="snrm")
            nc.scalar.activation(wsq, wtile, AF.Square, accum_out=wss)
            nc.scalar.sqrt(wss, wss)
            wps = warmp.tile([P, 4 * P], f32, tag="warm")
            for i in range(N_WARM):
                nc.tensor.matmul(
                    wps[:, (i % 4) * P:(i % 4 + 1) * P], wtile, wtile,
                    start=True, stop=True)

            # ---- prologue DMAs ----
            xs_s = [None] * ST

            def s_dma(t):
                x = loads.tile([P, C], bf16, tag="x_tile", name=f"sx{t}")
                nc.sync.dma_start(out=x, in_=sr_d[:, t * C:(t + 1) * C])
                xs_s[t] = x

            for t in range(4):
                s_dma(t)
            nc.sync.dma_start(out=ident, in_=ident_d)
            for j in range(KP):
                nc.sync.dma_start(
                    out=qT_sb[:, 2 * j:2 * j + 2, :],
                    in_=qT_d[:, 2 * j * PAD_P:(2 * j + 2) * PAD_P])
            nc.sync.dma_start(out=s8T[:, 2 * KP - 1, :], in_=zs_d)
            nc.sync.dma_start(out=ind_sb, in_=ind_d)
            qr_next = [0]

            def q_dma_ahead(upto):
                while qr_next[0] <= min(upto, MT - 1):
                    m = qr_next[0]
                    nc.sync.dma_start(
                        out=qr_sb[:, m * C:(m + 1) * C],
                        in_=qr_d[:, m * C:(m + 1) * C])
                    qr_next[0] += 1

            next_s = [4]

            def s_dma_ahead(upto):
                while next_s[0] <= min(upto, ST - 1):
                    s_dma(next_s[0])
                    next_s[0] += 1

            # ---- prep helpers ----
            def s_sq(t):
                sq = work.tile([P, C], bf16, tag="sq")
                nc.scalar.activation(sq, xs_s[t], AF.Square,
                                     accum_out=ssum_all[:, t:t + 1])

            def s_group_finish(g):
                # tiles 4g..4g+3: snrm = sqrt(ssum)/alpha ; sinv = alpha/|s|
                sl = slice(4 * g, 4 * g + 4)
                snrm = small.tile([P, 4], f32, tag="snrm", name=f"snrm{g}")
                nc.scalar.activation(snrm, ssum_all[:, sl], AF.Sqrt,
                                     scale=float(1.0 / (ALPHA * ALPHA)))
                nc.vector.reciprocal(sinv_all[:, sl], snrm)

            def s_mul_transpose_evict(t, mul_on_dve, evict_on_dve):
                x = xs_s[t]
                s_n = work.tile([P, C], bf16, tag="s_n", name=f"sn{t}")
                if mul_on_dve:
                    nc.vector.tensor_scalar_mul(s_n, x, sinv_all[:, t:t + 1])
                else:
                    nc.scalar.mul(s_n, x, sinv_all[:, t:t + 1])
                psA = tpp.tile([P, KC * P], bf16, tag="tp", name=f"psA{t}")
                for c in range(KC):
                    nc.tensor.transpose(
                        psA[:, c * P:(c + 1) * P], s_n[:, c * P:(c + 1) * P],
                        ident)
                dst = s8T[:, 0:KC, t * P:(t + 1) * P]
                src = psA.rearrange("p (c n) -> p c n", c=KC)
                if evict_on_dve:
                    nc.vector.tensor_copy(dst, src)
                else:
                    nc.scalar.copy(dst, src)

            def q_sq(m):
                sq = work.tile([P, C], bf16, tag="sq")
                nc.scalar.activation(sq, qr_sb[:, m * C:(m + 1) * C],
                                     AF.Square,
                                     accum_out=qsum_all[:, m:m + 1])

            # ---- prologue prep: support tiles 0-3 ----
            s_dma_ahead(7)
            for t in range(4):
                s_sq(t)
            s_group_finish(0)
            for t in range(4):
                s_mul_transpose_evict(t, mul_on_dve=True,
                                      evict_on_dve=(t % 2 == 1))
            q_dma_ahead(3)

            # q-square schedule: 4 per pass at these m slots (pass<4)
            QSQ_SLOTS = (3, 5, 9, 13)

            mxs = [None] * MT
            prev = [None, None]
            for w in range(WAY):
                for m in range(MT):
                    if w < 4:
                        t_new = 4 * (w + 1)
                        if m == 0:
                            s_dma_ahead(t_new + 3)
                        if m in (0, 2, 4, 6):
                            s_sq(t_new + m // 2)
                        if m == 7:
                            s_group_finish(w + 1)
                        if m in (8, 10, 12, 14):
                            t = t_new + (m - 8) // 2
                            s_mul_transpose_evict(
                                t, mul_on_dve=(t % 4 == 3),
                                evict_on_dve=(t % 4 == 1))
                        if m in QSQ_SLOTS:
                            mq = 4 * w + QSQ_SLOTS.index(m)
                            if mq < MT:
                                q_dma_ahead(mq + 2)
                                q_sq(mq)
                    if w == 4 and m == 0:
                        # qinv = 1 / (k * alpha * |q_patch|)
                        kn = small.tile([P, MT], f32, tag="kn")
                        nc.scalar.activation(
                            kn, qsum_all, AF.Sqrt,
                            scale=float(k * k * ALPHA * ALPHA))
                        nc.vector.reciprocal(qinv, kn)
                    if w == 0:
                        mxs[m] = mxp.tile([P, WAY * 8], f32, tag="mx",
                                          name=f"mx{m}")
                    psc = spp.tile([P, NW], f32, tag="psc",
                                   name=f"psc{m}_{w}")
                    for j in range(KP):
                        nc.tensor.matmul(
                            psc,
                            qT_sb[:, 2 * j:2 * j + 2, m * P:(m + 1) * P],
                            s8T[:, 2 * j:2 * j + 2, w * NW:(w + 1) * NW],
                            start=(j == 0),
                            stop=(j == KP - 1),
                            perf_mode=DR,
                        )
                    nc.vector.max(mxs[m][:, w * 8:(w + 1) * 8], psc)
                    if w == WAY - 1:
                        tsum = small.tile([P, WAY], f32, tag="tsum")
                        nc.vector.tensor_reduce(
                            tsum,
                            mxs[m].rearrange("p (w j) -> p w j", w=WAY)[:, :, :k],
                            axis=mybir.AxisListType.X,
                            op=mybir.AluOpType.add,
                        )
                        scaled = small.tile([P, WAY], f32, tag="scaled")
                        nc.scalar.mul(scaled, tsum, qinv[:, m:m + 1])
                        if prev[0] is not None:
                            nc.tensor.matmul(
                                out_ps,
                                ind_sb[:, prev[1] * NQ:(prev[1] + 1) * NQ],
                                prev[0], start=(prev[1] == 0), stop=False)
                        prev = [scaled, m]
            nc.tensor.matmul(
                out_ps, ind_sb[:, prev[1] * NQ:(prev[1] + 1) * NQ],
                prev[0], start=False, stop=True)
            out_sb = small.tile([NQ, WAY], f32, tag="out_sb")
            nc.scalar.copy(out_sb, out_ps)
            nc.sync.dma_start(out=out_d, in_=out_sb)

    nc.compile()
    return nc


def get_program(k: int):
    if k not in _prog_cache:
        _prog_cache[k] = _build(k)
    return _prog_cache[k]


def make_in_maps(input1: np.ndarray, input2: np.ndarray):
    """Shard full inputs into per-core input maps (layout + dtype only)."""
    input1 = np.ascontiguousarray(np.asarray(input1), dtype=np.float32)
    input2 = np.ascontiguousarray(np.asarray(input2), dtype=np.float32)
    fp8 = ml_dtypes.float8_e4m3fn
    bf16 = ml_dtypes.bfloat16
    in_maps = []
    for core in range(N_CORES):
        b = core // 4
        qs = (core % 4) * NQ
        qe = min(Q, qs + NQ)
        nq = qe - qs
        qdat = input1[b].reshape(Q, HW, C)[qs:qe].reshape(-1, C)
        qfull = np.ones((PAD_P, C), np.float32)
        qfull[: nq * HW] = qdat
        # qT: fp8 banded transpose [128, 6, PAD_P] (band 5 zeros)
        q8 = qfull.astype(fp8)
        qT = np.zeros((P, 2 * KP, PAD_P), fp8)
        for c in range(KC):
            qT[:, c, :] = q8[:, c * P:(c + 1) * P].T
        # q row-major bf16, partition-major tiles [128, MT, C]
        qr = np.ascontiguousarray(
            qfull.reshape(MT, P, C).transpose(1, 0, 2)).astype(bf16)
        sfull = np.ones((PAD_S, C), np.float32)
        sfull[:NS] = input2[b].reshape(NS, C)
        sr = np.ascontiguousarray(
            sfull.reshape(ST, P, C).transpose(1, 0, 2)).astype(bf16)
        # indicator: patch row p of M-tile t belongs to query (t*128+p)//HW
        ind = np.zeros((P, MT * NQ), np.float32)
        g = np.arange(MT * P)
        j = g // HW
        valid = j < nq
        ind[g[valid] % P, (g[valid] // P) * NQ + j[valid]] = 1.0
        in_maps.append({
            "qT": qT.reshape(P, 2 * KP * PAD_P),
            "qr": qr.reshape(P, MT * C),
            "sr": sr.reshape(P, ST * C),
            "zs": np.zeros((P, PAD_S), fp8),
            "ind": ind,
            "ident": np.eye(P).astype(bf16),
        })
    return in_maps


def gather_out(results) -> np.ndarray:
    out = np.zeros((B, Q, WAY), np.float32)
    for core in range(N_CORES):
        b = core // 4
        qs = (core % 4) * NQ
        n = min(Q, qs + NQ) - qs
        out[b, qs:qs + n] = results[core]["out"][:n]
    return out


def kernel(input1, input2, neighbor_k):
    k = int(np.asarray(neighbor_k))
    nc = get_program(k)
    in_maps = make_in_maps(input1, input2)
    # the axon-tunneled device occasionally reports a transient
    # "unrecoverable" state right after a previous process's teardown;
    # it recovers within seconds, so retry a couple of times
    import time
    last = None
    for attempt in range(3):
        try:
            res = run_bass_kernel_spmd(
                nc, in_maps, core_ids=list(range(N_CORES)))
            return gather_out(res.results)
        except Exception as e:  # noqa: BLE001
            last = e
            if attempt < 2:
                time.sleep(20.0 * (attempt + 1))
    raise last


# revision 14
# speedup vs baseline: 1.5107x; 1.0244x over previous
"""TRN2 Bass kernel for nn_MetaBaseline (DN4-style local-descriptor kNN).

Reference computation (per batch b):
  q = input1[b].reshape(7500, 640)           # query patch descriptors
  s = normalize(input2[b].reshape(2500, 640), axis=-1)
  scores = q @ s.T                           # [7500, 2500]
  per way group g (columns [500g, 500g+500)): top-k per row, mean,
  divide by |q_patch| (commutes with top-k since it is per-row), then
  sum over the 100 patches of each query -> out [75, 5].

Sharding: data-parallel over (b, query-quarter): 8 cores, each handles one
batch's quarter of queries (19 queries padded) with that batch's full
support replicated.

Implementation highlights vs the fp32r baseline:
- Score matmuls run in fp8e4m3 with MatmulPerfMode.DoubleRow (two 128-row
  k-tiles per pass): C=640 is padded to 6 chunks of 128 (band 5 zeros) so
  each (patch-tile, way) pair needs just 3 PE instructions.
- The query operand is uploaded pre-transposed in fp8 from the host
  (layout + dtype conversion only), so queries need no PE transpose, no
  PSUM eviction and no quantize pass on device. A second row-major bf16
  copy of q feeds the |q_patch| norm pass (ACT square+accum), matching the
  reference's exact-norm division.
- Support arrives row-major bf16; on device: ACT square+accum -> batched
  sqrt(scale=1/alpha^2) -> DVE reciprocal gives sinv = alpha/|s| per
  descriptor; DVE tensor_scalar (4x mode on bf16) or ACT scales to
  alpha-normalized bf16; PE transposes (bf16 identity, 1 cycle/row) into
  one PSUM bank per tile; a single strided copy evicts+converts to the
  fp8 banded layout. alpha=32 keeps e4m3 operands in the normal range;
  1/alpha folds into the per-query scale sqrt(k^2 alpha^2 sum q^2).
- Top-8 per (patch, way) via DVE max8 straight from the PSUM score bank;
  pass 4 finishes each patch tile with a strided top-k tensor_reduce, an
  ACT scale by 1/(k*alpha*|q_patch|), and a small fp32 indicator matmul
  accumulating per-query sums in PSUM -> [19, 5].
"""
import os
from contextlib import ExitStack

import ml_dtypes
import numpy as np

import concourse.bass as bass  # noqa: F401
import concourse.mybir as mybir
import concourse.tile as tile
from concourse import bacc
from concourse.bass_utils import run_bass_kernel_spmd

# Problem geometry (hardcoded per contest rules)
B, Q, WAY, SHOT, H, W, C = 2, 75, 5, 5, 10, 10, 640
HW = H * W               # 100 patches per query / support image
NQ = 19                  # queries per core (4 cores x 19 = 76 >= 75)
MT = 15                  # patch M-tiles of 128 -> 1920 rows (1900 real)
PAD_P = MT * 128
NS = WAY * SHOT * HW     # 2500 support descriptors per batch
ST = 20                  # support tiles of 128 -> 2560 rows
PAD_S = ST * 128
KC = 5                   # real C chunks of 128 (640 = 5*128)
KP = 3                   # DoubleRow chunk pairs (6 bands incl. zero band)
P = 128
NW = SHOT * HW           # 500 support descriptors per way group
N_CORES = 8
ALPHA = 32.0             # support operand scale for fp8 normal range
N_WARM = int(os.environ.get("N_WARM", "18"))
# NOTE: tensor_tensor_reduce with bf16 inputs faults on real TRN2 hardware
# (verified by bisection); squares therefore run on ACT only.
SQ_DVE = os.environ.get("SQ_DVE", "0") == "1"   # squares split ACT/DVE
TPP_BUFS = int(os.environ.get("TPP", "3"))       # transpose PSUM banks
SCB = os.environ.get("SCB", "1") == "1"          # scaled on DVE bf16 + bf16 ind

_prog_cache: dict[int, object] = {}


def _build(k: int):
    """Build + compile the per-core SPMD program for neighbor_k == k."""
    assert 1 <= k <= 8, f"neighbor_k={k} not supported (need 1..8)"
    nc = bacc.Bacc("TRN2", target_bir_lowering=False, debug=False)
    f32 = mybir.dt.float32
    bf16 = mybir.dt.bfloat16
    fp8 = mybir.dt.float8e4
    AF = mybir.ActivationFunctionType
    DR = mybir.MatmulPerfMode.DoubleRow

    qT_d = nc.dram_tensor("qT", [P, 2 * KP * PAD_P], fp8,
                          kind="ExternalInput").ap()
    qr_d = nc.dram_tensor("qr", [P, MT * C], bf16, kind="ExternalInput").ap()
    sr_d = nc.dram_tensor("sr", [P, ST * C], bf16, kind="ExternalInput").ap()
    zs_d = nc.dram_tensor("zs", [P, PAD_S], fp8, kind="ExternalInput").ap()
    ind_dt = bf16 if SCB else f32
    ind_d = nc.dram_tensor("ind", [P, MT * NQ], ind_dt,
                           kind="ExternalInput").ap()
    ident_d = nc.dram_tensor("ident", [P, P], bf16, kind="ExternalInput").ap()
    out_d = nc.dram_tensor("out", [NQ, WAY], f32, kind="ExternalOutput").ap()

    with tile.TileContext(nc) as tc:
        with ExitStack() as ctx:
            const = ctx.enter_context(tc.tile_pool(name="const", bufs=1))
            big = ctx.enter_context(tc.tile_pool(name="big", bufs=1))
            loads = ctx.enter_context(tc.tile_pool(name="loads", bufs=8))
            work = ctx.enter_context(tc.tile_pool(name="work", bufs=3))
            small = ctx.enter_context(tc.tile_pool(name="small", bufs=4))
            mxp = ctx.enter_context(tc.tile_pool(name="mxp", bufs=MT))
            outp = ctx.enter_context(
                tc.tile_pool(name="outp", bufs=1, space="PSUM")
            )
            tpp = ctx.enter_context(
                tc.tile_pool(name="tpp", bufs=TPP_BUFS, space="PSUM")
            )
            warmp = ctx.enter_context(
                tc.tile_pool(name="warmp", bufs=1, space="PSUM")
            )
            spp = ctx.enter_context(
                tc.tile_pool(name="spp", bufs=3, space="PSUM")
            )

            ident = const.tile([P, P], bf16)
            ind_sb = const.tile([P, MT * NQ], ind_dt)
            # banded transposed operands: [partition, chunk band, column]
            s8T = big.tile([P, 2 * KP, PAD_S], fp8, name="s8T")
            qT_sb = big.tile([P, 2 * KP, PAD_P], fp8, name="qT_sb")
            qr_sb = big.tile([P, MT * C], bf16, name="qr_sb")

            # batched norm scalars
            ssum_all = const.tile([P, ST], f32, name="ssum")
            sinv_all = const.tile([P, ST], f32, name="sinv")
            qsum_all = const.tile([P, MT], f32, name="qsum")
            qinv = const.tile([P, MT], f32, name="qinv")

            out_ps = outp.tile([NQ, WAY], f32)

            # ---- warmups: ACT tables + PE pipeline + pstate ramp ----
            wtile = const.tile([P, P], bf16, name="wtile")
            nc.vector.memset(wtile, 1.0)
            wsq = small.tile([P, P], bf16, tag="wsq")
            wss = small.tile([P, 1], f32, tag="snrm")
            nc.scalar.activation(wsq, wtile, AF.Square, accum_out=wss)
            nc.scalar.sqrt(wss, wss)
            wps = warmp.tile([P, 4 * P], f32, tag="warm")
            for i in range(N_WARM):
                nc.tensor.matmul(
                    wps[:, (i % 4) * P:(i % 4 + 1) * P], wtile, wtile,
                    start=True, stop=True)

            # ---- prologue DMAs ----
            xs_s = [None] * ST

            def s_dma(t):
                x = loads.tile([P, C], bf16, tag="x_tile", name=f"sx{t}")
                nc.sync.dma_start(out=x, in_=sr_d[:, t * C:(t + 1) * C])
                xs_s[t] = x

            for t in range(4):
                s_dma(t)
            nc.sync.dma_start(out=ident, in_=ident_d)
            for j in range(KP):
                nc.sync.dma_start(
                    out=qT_sb[:, 2 * j:2 * j + 2, :],
                    in_=qT_d[:, 2 * j * PAD_P:(2 * j + 2) * PAD_P])
            nc.sync.dma_start(out=s8T[:, 2 * KP - 1, :], in_=zs_d)
            nc.sync.dma_start(out=ind_sb, in_=ind_d)
            qr_next = [0]

            def q_dma_ahead(upto):
                while qr_next[0] <= min(upto, MT - 1):
                    m = qr_next[0]
                    nc.sync.dma_start(
                        out=qr_sb[:, m * C:(m + 1) * C],
                        in_=qr_d[:, m * C:(m + 1) * C])
                    qr_next[0] += 1

            next_s = [4]

            def s_dma_ahead(upto):
                while next_s[0] <= min(upto, ST - 1):
                    s_dma(next_s[0])
                    next_s[0] += 1

            # ---- prep helpers ----
            def s_sq(t, on_dve=False):
                if on_dve and SQ_DVE:
                    sqf = work.tile([P, C], f32, tag="sqf")
                    nc.vector.tensor_tensor_reduce(
                        sqf, xs_s[t], xs_s[t], 1.0, 0.0,
                        mybir.AluOpType.mult, mybir.AluOpType.add,
                        ssum_all[:, t:t + 1])
                else:
                    sq = work.tile([P, C], bf16, tag="sq")
                    nc.scalar.activation(sq, xs_s[t], AF.Square,
                                         accum_out=ssum_all[:, t:t + 1])

            def s_finish(t0, n):
                # tiles t0..t0+n-1: snrm = sqrt(ssum)/alpha ; sinv = alpha/|s|
                sl = slice(t0, t0 + n)
                snrm = small.tile([P, n], f32, tag="snrm", name=f"snrm{t0}")
                nc.scalar.activation(snrm, ssum_all[:, sl], AF.Sqrt,
                                     scale=float(1.0 / (ALPHA * ALPHA)))
                nc.vector.reciprocal(sinv_all[:, sl], snrm)

            def s_mul_transpose_evict(t, mul_on_dve, evict_on_dve):
                x = xs_s[t]
                s_n = work.tile([P, C], bf16, tag="s_n", name=f"sn{t}")
                if mul_on_dve:
                    nc.vector.tensor_scalar_mul(s_n, x, sinv_all[:, t:t + 1])
                else:
                    nc.scalar.mul(s_n, x, sinv_all[:, t:t + 1])
                psA = tpp.tile([P, KC * P], bf16, tag="tp", name=f"psA{t}")
                for c in range(KC):
                    nc.tensor.transpose(
                        psA[:, c * P:(c + 1) * P], s_n[:, c * P:(c + 1) * P],
                        ident)
                dst = s8T[:, 0:KC, t * P:(t + 1) * P]
                src = psA.rearrange("p (c n) -> p c n", c=KC)
                if evict_on_dve:
                    nc.vector.tensor_copy(dst, src)
                else:
                    nc.scalar.copy(dst, src)

            def q_sq(m):
                sq = work.tile([P, C], bf16, tag="sq")
                nc.scalar.activation(sq, qr_sb[:, m * C:(m + 1) * C],
                                     AF.Square,
                                     accum_out=qsum_all[:, m:m + 1])

            # ---- prologue prep: support tiles 0-3 ----
            # squares run pairwise on ACT+DVE so the sinv chain finishes in
            # two hops and PE transposes can start as early as possible
            s_dma_ahead(7)
            s_sq(0)
            s_sq(1, on_dve=True)
            s_finish(0, 2)
            s_sq(2)
            s_sq(3, on_dve=True)
            s_mul_transpose_evict(0, mul_on_dve=True, evict_on_dve=False)
            s_finish(2, 2)
            s_mul_transpose_evict(1, mul_on_dve=True, evict_on_dve=True)
            s_mul_transpose_evict(2, mul_on_dve=True, evict_on_dve=False)
            s_mul_transpose_evict(3, mul_on_dve=True, evict_on_dve=True)
            q_dma_ahead(3)

            # per-pass schedule slots (pass w<4 preps tiles 4w+4..4w+7):
            # squares early, norm finish, prep units with slack before the
            # pass boundary, q squares interleaved
            SQ_SLOTS = (0, 1, 2, 3)
            UNIT_SLOTS = (5, 7, 9, 11)
            QSQ_SLOTS = (6, 8, 10, 12)

            mxs = [None] * MT
            prev = [None, None]
            for w in range(WAY):
                for m in range(MT):
                    if w < 4:
                        t_new = 4 * (w + 1)
                        if m == 0:
                            s_dma_ahead(t_new + 7)
                        if m in SQ_SLOTS:
                            i = SQ_SLOTS.index(m)
                            s_sq(t_new + i, on_dve=(i % 2 == 1))
                        if m == 4:
                            s_finish(t_new, 4)
                        if m in UNIT_SLOTS:
                            t = t_new + UNIT_SLOTS.index(m)
                            s_mul_transpose_evict(
                                t, mul_on_dve=(t % 4 == 3),
                                evict_on_dve=(t % 4 == 1))
                        if m in QSQ_SLOTS:
                            mq = 4 * w + QSQ_SLOTS.index(m)
                            if mq < MT:
                                q_dma_ahead(mq + 2)
                                q_sq(mq)
                    if w == 4 and m == 0:
                        # qinv = 1 / (k * alpha * |q_patch|)
                        kn = small.tile([P, MT], f32, tag="kn")
                        nc.scalar.activation(
                            kn, qsum_all, AF.Sqrt,
                            scale=float(k * k * ALPHA * ALPHA))
                        nc.vector.reciprocal(qinv, kn)
                    if w == 0:
                        mxs[m] = mxp.tile([P, WAY * 8], f32, tag="mx",
                                          name=f"mx{m}")
                    psc = spp.tile([P, NW], f32, tag="psc",
                                   name=f"psc{m}_{w}")
                    for j in range(KP):
                        nc.tensor.matmul(
                            psc,
                            qT_sb[:, 2 * j:2 * j + 2, m * P:(m + 1) * P],
                            s8T[:, 2 * j:2 * j + 2, w * NW:(w + 1) * NW],
                            start=(j == 0),
                            stop=(j == KP - 1),
                            perf_mode=DR,
                        )
                    nc.vector.max(mxs[m][:, w * 8:(w + 1) * 8], psc)
                    if w == WAY - 1:
                        tsum = small.tile([P, WAY], f32, tag="tsum")
                        nc.vector.tensor_reduce(
                            tsum,
                            mxs[m].rearrange("p (w j) -> p w j", w=WAY)[:, :, :k],
                            axis=mybir.AxisListType.X,
                            op=mybir.AluOpType.add,
                        )
                        sc_dt = bf16 if SCB else f32
                        scaled = small.tile([P, WAY], sc_dt, tag="scaled")
                        if SCB:
                            nc.vector.tensor_scalar_mul(scaled, tsum,
                                                        qinv[:, m:m + 1])
                        else:
                            nc.scalar.mul(scaled, tsum, qinv[:, m:m + 1])
                        if prev[0] is not None:
                            nc.tensor.matmul(
                                out_ps,
                                ind_sb[:, prev[1] * NQ:(prev[1] + 1) * NQ],
                                prev[0], start=(prev[1] == 0), stop=False)
                        prev = [scaled, m]
            nc.tensor.matmul(
                out_ps, ind_sb[:, prev[1] * NQ:(prev[1] + 1) * NQ],
                prev[0], start=False, stop=True)
            out_sb = small.tile([NQ, WAY], f32, tag="out_sb")
            nc.scalar.copy(out_sb, out_ps)
            nc.sync.dma_start(out=out_d, in_=out_sb)

    nc.compile()
    return nc


def get_program(k: int):
    if k not in _prog_cache:
        _prog_cache[k] = _build(k)
    return _prog_cache[k]


def make_in_maps(input1: np.ndarray, input2: np.ndarray):
    """Shard full inputs into per-core input maps (layout + dtype only)."""
    input1 = np.ascontiguousarray(np.asarray(input1), dtype=np.float32)
    input2 = np.ascontiguousarray(np.asarray(input2), dtype=np.float32)
    fp8 = ml_dtypes.float8_e4m3fn
    bf16 = ml_dtypes.bfloat16
    in_maps = []
    for core in range(N_CORES):
        b = core // 4
        qs = (core % 4) * NQ
        qe = min(Q, qs + NQ)
        nq = qe - qs
        qdat = input1[b].reshape(Q, HW, C)[qs:qe].reshape(-1, C)
        qfull = np.ones((PAD_P, C), np.float32)
        qfull[: nq * HW] = qdat
        # qT: fp8 banded transpose [128, 6, PAD_P] (band 5 zeros)
        q8 = qfull.astype(fp8)
        qT = np.zeros((P, 2 * KP, PAD_P), fp8)
        for c in range(KC):
            qT[:, c, :] = q8[:, c * P:(c + 1) * P].T
        # q row-major bf16, partition-major tiles [128, MT, C]
        qr = np.ascontiguousarray(
            qfull.reshape(MT, P, C).transpose(1, 0, 2)).astype(bf16)
        sfull = np.ones((PAD_S, C), np.float32)
        sfull[:NS] = input2[b].reshape(NS, C)
        sr = np.ascontiguousarray(
            sfull.reshape(ST, P, C).transpose(1, 0, 2)).astype(bf16)
        # indicator: patch row p of M-tile t belongs to query (t*128+p)//HW
        ind = np.zeros((P, MT * NQ), np.float32)
        g = np.arange(MT * P)
        j = g // HW
        valid = j < nq
        ind[g[valid] % P, (g[valid] // P) * NQ + j[valid]] = 1.0
        in_maps.append({
            "qT": qT.reshape(P, 2 * KP * PAD_P),
            "qr": qr.reshape(P, MT * C),
            "sr": sr.reshape(P, ST * C),
            "zs": np.zeros((P, PAD_S), fp8),
            "ind": ind.astype(bf16) if SCB else ind,
            "ident": np.eye(P).astype(bf16),
        })
    return in_maps


def gather_out(results) -> np.ndarray:
    out = np.zeros((B, Q, WAY), np.float32)
    for core in range(N_CORES):
        b = core // 4
        qs = (core % 4) * NQ
        n = min(Q, qs + NQ) - qs
        out[b, qs:qs + n] = results[core]["out"][:n]
    return out


def kernel(input1, input2, neighbor_k):
    k = int(np.asarray(neighbor_k))
    nc = get_program(k)
    in_maps = make_in_maps(input1, input2)
    # the axon-tunneled device occasionally reports a transient
    # "unrecoverable" state right after a previous process's teardown;
    # it recovers within seconds, so retry a couple of times
    import time
    last = None
    for attempt in range(3):
        try:
            res = run_bass_kernel_spmd(
                nc, in_maps, core_ids=list(range(N_CORES)))
            return gather_out(res.results)
        except Exception as e:  # noqa: BLE001
            last = e
            if attempt < 2:
                time.sleep(20.0 * (attempt + 1))
    raise last
